# revision 24
# baseline (speedup 1.0000x reference)
"""Gaussian covariance kernel for Trainium2 (8 NeuronCores, SPMD).

Computes, per gaussian n:
    s = exp(scale[n])                  # [3]
    q = rot[n] / ||rot[n]||            # [4] quaternion (r,i,j,k)
    R = quat_to_rotmat(q)              # [3,3]
    Sigma[n] = (R*s) @ (R*s)^T         # [3,3]

Inputs : scale [4_000_000, 3] f32, rot [4_000_000, 4] f32
Output : [4_000_000, 3, 3] f32

Sharding: data-parallel over the gaussian dim across 8 cores
(500_000 each, padded to 500_096 = 128*3907 per core).

Math (scale-invariant, avoids the normalize):
    n2 = |q|^2 (unnormalized); K = quat formula without the 1/n2
    (diag entries carry the n2, offdiag entries are UNdoubled: k' = K/2)
    w'_j  = exp(s_j - ln n2)           -> B_jj   = K_jj * w'_j
    w2'_j = 2 w'_j                     -> B_ij   = k'_ij * w2'_j (i != j)
    Sigma = B @ B^T

Precision: inputs are downcast to bf16 on the host (halves input DMA),
the n2 -> ln -> exp-argument chain runs in fp32 on device, everything
else (products, K, B, Sigma) runs in bf16 (DVE 2x mode needs packed
stride-1 16-bit operands, hence the component-major [P, C, L] layout).
Output is written as bf16 and upcast to fp32 on the host.
Validated end-to-end in numpy: L2 rel err 6.5e-3 (gate 2e-2).
"""

import numpy as np
import ml_dtypes

N_TOTAL = 4_000_000
N_CORES = 8
N_PER_CORE = N_TOTAL // N_CORES          # 500_000
P = 128
L = 3907                                 # ceil(500_000/128) -> pad to 128*3907
N_PAD = P * L                            # 500_096
F_TILE = 512                             # gaussians per partition per tile

BF16 = ml_dtypes.bfloat16
LN2 = 0.6931471805599453

_STATE = {}


def _build_kernel(L=L, F_TILE=F_TILE, pool_b=6, pool_tm=True, pool_sums=True,
                  pool_prod=3, bufs=2, pool_sq=True, pool_kp=True):
    import concourse.bass as bass
    import concourse.bacc as bacc
    import concourse.tile as tile
    from concourse import mybir

    f32 = mybir.dt.float32
    bf16 = mybir.dt.bfloat16
    Alu = mybir.AluOpType
    Act = mybir.ActivationFunctionType

    nc = bacc.Bacc("TRN2", target_bir_lowering=False, debug=False,
                   num_devices=N_CORES)

    rot_d = nc.dram_tensor("rot", [P, 4 * L], bf16, kind="ExternalInput").ap() \
        .rearrange("p (c l) -> p c l", c=4)
    scl_d = nc.dram_tensor("scale", [P, 3 * L], bf16, kind="ExternalInput").ap() \
        .rearrange("p (c l) -> p c l", c=3)
    out_d = nc.dram_tensor("out", [P, 9 * L], bf16, kind="ExternalOutput").ap() \
        .rearrange("p (c l) -> p c l", c=9)

    bounds = []
    t0 = 0
    while t0 < L:
        f = min(F_TILE, L - t0)
        bounds.append((t0, f))
        t0 += f

    # Force a single activation table: {Ln, Exp, Square, Copy} all live in
    # act_info.json's "natural_log_exp_and_others" set, but the default
    # table-assignment pass picks per-func first-match tables and ping-pongs
    # Ln/Exp loads every tile (~1.3us each).  Emptying every other set (list
    # positions kept, so act_func_set_id indices stay valid) pins all
    # activations to the one table -> a single load at kernel start.
    import concourse.bacc as bacc_mod
    from concourse.hw_specs import get_activation_tables

    def _patched_insert_act_table_loads():
        has_activation = any(
            isinstance(i, mybir.InstActivation)
            for b in nc.main_func.blocks
            for i in b.instructions
        )
        if not has_activation:
            return
        keep = "natural_log_exp_and_others"
        tables = [
            (nm, (s if nm == keep else set()))
            for nm, s in get_activation_tables(nc.m.arch).items()
        ]
        assert any(nm == keep and s for nm, s in tables)
        bacc_mod._bass_rust.insert_act_table_loads(nc, tables)

    nc.insert_act_table_loads = _patched_insert_act_table_loads

    with tile.TileContext(nc) as tc, \
            nc.allow_low_precision("bf16 kernel, harness gate is 2e-2"):
        with tc.tile_pool(name="io", bufs=bufs) as io, \
             tc.tile_pool(name="tmp", bufs=bufs) as tp:
            for (t0, F) in bounds:
                rot_t = io.tile([P, 4, F], bf16, tag="rot")
                scl_t = io.tile([P, 3, F], bf16, tag="scl")
                out_t = io.tile([P, 9, F], bf16, tag="out")
                nc.sync.dma_start(out=rot_t, in_=rot_d[:, :, t0:t0 + F])
                nc.sync.dma_start(out=scl_t, in_=scl_d[:, :, t0:t0 + F])

                qr = rot_t[:, 0, :]
                qi = rot_t[:, 1, :]
                qj = rot_t[:, 2, :]
                qk = rot_t[:, 3, :]

                # ---- fp32 side: n2 and the exp-argument chain ----
                # squares of the (bf16-rounded) quaternion, in fp32
                sq_t = tp.tile([P, 4, F], f32, tag="sq")
                (nc.gpsimd if pool_sq else nc.vector).tensor_mul(
                    out=sq_t, in0=rot_t, in1=rot_t)
                d_ = sq_t[:, 0, :]
                a_ = sq_t[:, 1, :]
                b_ = sq_t[:, 2, :]
                c_ = sq_t[:, 3, :]

                # pair sums: pr rows = (b+c, a+c, a+b) feed the K diagonal
                ad = tp.tile([P, F], f32, tag="ad")
                pr = tp.tile([P, 3, F], f32, tag="pr")
                n2 = tp.tile([P, F], f32, tag="n2")
                se = nc.gpsimd if pool_sums else nc.vector
                se.tensor_add(out=ad, in0=d_, in1=a_)
                se.tensor_add(out=pr[:, 0, :], in0=b_, in1=c_)
                se.tensor_add(out=pr[:, 1, :], in0=a_, in1=c_)
                se.tensor_add(out=pr[:, 2, :], in0=a_, in1=b_)
                se.tensor_add(out=n2, in0=ad, in1=pr[:, 0, :])

                # K diagonal: K_jj = n2 - 2*pr_j  -> bf16 (STT is DVE-only)
                kd = tp.tile([P, 3, F], bf16, tag="kd")
                nc.vector.scalar_tensor_tensor(
                    out=kd, in0=pr, scalar=-2.0,
                    in1=n2.rearrange("p (c f) -> p c f", c=1)
                         .broadcast_to([P, 3, F]),
                    op0=Alu.mult, op1=Alu.add)

                # exp arguments: tm_j = s_j - ln(n2), fp32
                lg = tp.tile([P, F], f32, tag="lg")
                nc.scalar.activation(out=lg, in_=n2, func=Act.Ln)
                tm = tp.tile([P, 3, F], f32, tag="tm")
                (nc.gpsimd if pool_tm else nc.vector).tensor_sub(
                    out=tm, in0=scl_t,
                    in1=lg.rearrange("p (c f) -> p c f", c=1)
                         .broadcast_to([P, 3, F]))

                # w' = exp(tm) on ACT; w2' = 2*w' on DVE (bf16 tensor_scalar
                # hits the 4x perf mode, ~0.26ns/elem)
                wp = tp.tile([P, 3, F], bf16, tag="wp")
                wp2 = tp.tile([P, 3, F], bf16, tag="wp2")
                nc.scalar.activation(out=wp.rearrange("p c f -> p (c f)"),
                                     in_=tm.rearrange("p c f -> p (c f)"),
                                     func=Act.Exp)
                nc.vector.tensor_scalar_mul(
                    out=wp2.rearrange("p c f -> p (c f)"),
                    in0=wp.rearrange("p c f -> p (c f)"), scalar1=2.0)

                # ---- bf16 side: products, off-diag K, B, Sigma ----
                # pa rows = (qi*qj, qi*qk, qj*qk); pb rows = (qk*qr, qj*qr, qi*qr)
                pa = tp.tile([P, 3, F], bf16, tag="pa")
                pb = tp.tile([P, 3, F], bf16, tag="pb")
                prod_list = [(pa, 0, qi, qj), (pa, 1, qi, qk), (pa, 2, qj, qk),
                             (pb, 0, qk, qr), (pb, 1, qj, qr), (pb, 2, qi, qr)]
                for n_, (dst, row, x, y) in enumerate(prod_list):
                    eng = nc.gpsimd if n_ < pool_prod else nc.vector
                    eng.tensor_mul(out=dst[:, row, :], in0=x, in1=y)

                # k' = K/2 off-diagonals: kp rows = (k10, k02, k21),
                # km rows = (k01, k20, k12)
                kp = tp.tile([P, 3, F], bf16, tag="kp")
                km = tp.tile([P, 3, F], bf16, tag="km")
                (nc.gpsimd if pool_kp else nc.vector).tensor_add(
                    out=kp.rearrange("p c f -> p (c f)"),
                    in0=pa.rearrange("p c f -> p (c f)"),
                    in1=pb.rearrange("p c f -> p (c f)"))
                nc.vector.tensor_sub(out=km.rearrange("p c f -> p (c f)"),
                                     in0=pa.rearrange("p c f -> p (c f)"),
                                     in1=pb.rearrange("p c f -> p (c f)"))

                # B matrix, stored row-major [P, 3(row i), 3(col j), F]
                B = tp.tile([P, 3, 3, F], bf16, tag="B")
                B9 = B.rearrange("p i j f -> p (i j) f")
                # diagonal (channels 0,4,8): B_jj = K_jj * w'_j, one batched op
                nc.vector.tensor_mul(out=B9[:, 0:9:4, :], in0=kd, in1=wp)
                # off-diagonal: B_ij = k'_ij * w2'_j (pool_b of them on Pool)
                boff = [(1, 0, kp, 0, 0), (0, 1, km, 0, 1), (0, 2, kp, 1, 2),
                        (2, 0, km, 1, 0), (2, 1, kp, 2, 1), (1, 2, km, 2, 2)]
                for n_, (i, j, src, row, wrow) in enumerate(boff):
                    eng = nc.gpsimd if n_ < pool_b else nc.vector
                    eng.tensor_mul(out=B[:, i, j, :], in0=src[:, row, :],
                                   in1=wp2[:, wrow, :])

                # Sigma = B @ B^T.
                # Output channel order (host permutes back):
                #   ch 0..2 = S00,S11,S22 ; ch 3..5 = S01,S02,S12 ;
                #   ch 6..8 = S10,S20,S21 (copies of 3..5)
                # Diagonal via ACT squares: S_ii = sum_j B_ij^2
                sqB = tp.tile([P, 3, 3, F], bf16, tag="sqB")
                nc.scalar.activation(
                    out=sqB.rearrange("p i j f -> p (i j f)"),
                    in_=B.rearrange("p i j f -> p (i j f)"),
                    func=Act.Square)
                dd = tp.tile([P, 3, F], bf16, tag="dd")
                nc.vector.tensor_add(out=dd, in0=sqB[:, :, 0, :],
                                     in1=sqB[:, :, 1, :])
                nc.vector.tensor_add(out=out_t[:, 0:3, :], in0=dd,
                                     in1=sqB[:, :, 2, :])
                # Off-diagonal: T[p] = B_row_i * B_row_k elementwise, then sum
                T = tp.tile([P, 3, 3, F], bf16, tag="T")
                for p, (i, k) in enumerate([(0, 1), (0, 2), (1, 2)]):
                    nc.vector.tensor_mul(out=T[:, p, :, :], in0=B[:, i, :, :],
                                         in1=B[:, k, :, :])
                os_ = tp.tile([P, 3, F], bf16, tag="os")
                nc.vector.tensor_add(out=os_, in0=T[:, :, 0, :],
                                     in1=T[:, :, 1, :])
                nc.vector.tensor_add(out=out_t[:, 3:6, :], in0=os_,
                                     in1=T[:, :, 2, :])
                # symmetric lower entries, one batched ACT copy
                nc.scalar.copy(out=out_t[:, 6:9, :], in_=out_t[:, 3:6, :])

                nc.sync.dma_start(out=out_d[:, :, t0:t0 + F], in_=out_t)

    nc.compile()
    return nc


def _get_nc():
    if "nc" not in _STATE:
        _STATE["nc"] = _build_kernel()
    return _STATE["nc"]


def kernel(scale: np.ndarray, rot: np.ndarray) -> np.ndarray:
    from concourse.bass_utils import run_bass_kernel_spmd

    scale = np.asarray(scale, dtype=np.float32)
    rot = np.asarray(rot, dtype=np.float32)

    nc = _get_nc()

    in_maps = []
    for c in range(N_CORES):
        s = np.zeros((N_PAD, 3), np.float32)
        s[:N_PER_CORE] = scale[c * N_PER_CORE:(c + 1) * N_PER_CORE]
        r = np.zeros((N_PAD, 4), np.float32)
        r[:N_PER_CORE] = rot[c * N_PER_CORE:(c + 1) * N_PER_CORE]
        r[N_PER_CORE:, 0] = 1.0
        # component-major [P, C, L] layout, bf16
        sb = np.ascontiguousarray(
            s.reshape(P, L, 3).transpose(0, 2, 1)).astype(BF16)
        rb = np.ascontiguousarray(
            r.reshape(P, L, 4).transpose(0, 2, 1)).astype(BF16)
        in_maps.append({
            "scale": sb.reshape(P, 3 * L),
            "rot": rb.reshape(P, 4 * L),
        })

    res = run_bass_kernel_spmd(nc, in_maps, core_ids=list(range(N_CORES)))

    # device channel order: [S00,S11,S22,S01,S02,S12,S10,S20,S21]
    # flat (i*3+k) position -> device channel
    perm = [0, 3, 4, 6, 1, 5, 7, 8, 2]
    out = np.empty((N_TOTAL, 9), np.float32)
    for c in range(N_CORES):
        o = res.results[c]["out"].reshape(P, 9, L)[:, perm, :].transpose(0, 2, 1)
        out[c * N_PER_CORE:(c + 1) * N_PER_CORE] = (
            o.reshape(N_PAD, 9)[:N_PER_CORE].astype(np.float32))
    return out.reshape(N_TOTAL, 3, 3)


# revision 27
# speedup vs baseline: 1.3140x; 1.3140x over previous
"""Gaussian covariance kernel for Trainium2 (8 NeuronCores, SPMD).

Computes, per gaussian n:
    s = exp(scale[n])                  # [3]
    q = rot[n] / ||rot[n]||            # [4] quaternion (r,i,j,k)
    R = quat_to_rotmat(q)              # [3,3]
    Sigma[n] = (R*s) @ (R*s)^T         # [3,3]

Inputs : scale [4_000_000, 3] f32, rot [4_000_000, 4] f32
Output : [4_000_000, 3, 3] f32

Sharding: data-parallel over the gaussian dim across 8 cores
(500_000 each, padded to 500_096 = 128*3907 per core).

Math (scale-invariant, no normalize; everything at K/2 scale):
    n2   = |q|^2 (unnormalized)
    Khat = K/2 where K is the unnormalized rotation numerator:
           Khat_jj  = n2/2 - pair_j      (pair = (b+c, a+c, a+b))
           Khat_off = ij +- kr etc (UNdoubled products)
    wp_j = exp(s_j - ln n2)
    B    = Khat * diag(wp)  (single broadcast multiply, column-uniform)
    Sigma/4 = B @ B^T       (host multiplies the decoded output by 4)

Precision: bf16 inputs (host cast, halves input DMA), fp32 n2/ln/exp
chain on device, bf16 everywhere else, bf16 output (host upcasts and
multiplies by 4 -- exact power-of-two scaling). End-to-end L2 rel err
6.5e-3 (gate 2e-2).

Layout: component-major [P, C, L] so every op sees packed stride-1
innermost APs.  All per-component ops are batched into multi-row
instructions via affine row tricks (HW shows ~0.5-1.2us fixed cost per
DVE/Pool instruction, so instruction count dominates).
  K tile is column-major [P, 3(col j), 3(row i), F]; flat slot = 3j+i.
  kp rows (k10,k02,k21) -> slots (1,6,5): split (1,6) step5 + (5)
  km rows (k01,k20,k12) -> slots (3,2,7): split (3,2) step-1 + (7)
  kd rows (K00,K11,K22) -> slots (0,4,8) step 4
  sums6 f32 rows = (ad, u, G1, G2, db, n2):
    V2: rows (0,1) = (d+a, b+c)   <- sq[0:3:2] + sq[1:4:2]
    V1: rows (4,2) = (d+b, a+c)   <- sq[0:2] + sq[2:4]  (db is a dead
        byproduct that buys the affine pairing)
    G2: row 3     = a+b ;  n2: row 5 = ad + u
  products: prodA = (ir,jr,kr) = rot[1:4] * bcast(rot[0])
            pa[0::2] = (ij, jk) = rot[1:3] * rot[2:4];  pa[1] = ik
            pb view  = prodA reversed rows = (kr, jr, ir)
  Sigma: U [P,6,3,F] holds sqB (rows 0-2, as [i][j]) and T (rows 3-5,
  pair p = (0,1),(0,2),(1,2)); two batched adds produce out chans 0..5 =
  (S00,S11,S22,S01,S02,S12); ACT copies chans 3..5 -> 6..8.
"""

import numpy as np
import ml_dtypes

N_TOTAL = 4_000_000
N_CORES = 8
N_PER_CORE = N_TOTAL // N_CORES          # 500_000
P = 128
L = 3907                                 # ceil(500_000/128) -> pad to 128*3907
N_PAD = P * L                            # 500_096
F_TILE = 512

BF16 = ml_dtypes.bfloat16

_STATE = {}


def _build_kernel(L=L, F_TILE=F_TILE, bufs=2,
                  dve=("kd", "tm", "rmul", "T0", "T1", "T2", "ddall",
                       "finall", "P1", "kpA", "kmA"),
                  act_sq=True):
    import concourse.bass as bass
    import concourse.bacc as bacc
    import concourse.tile as tile
    from concourse import mybir

    f32 = mybir.dt.float32
    bf16 = mybir.dt.bfloat16
    Alu = mybir.AluOpType
    Act = mybir.ActivationFunctionType

    nc = bacc.Bacc("TRN2", target_bir_lowering=False, debug=False,
                   num_devices=N_CORES)

    def eng(name):
        return nc.vector if name in dve else nc.gpsimd

    rot_d = nc.dram_tensor("rot", [P, 4 * L], bf16, kind="ExternalInput").ap() \
        .rearrange("p (c l) -> p c l", c=4)
    scl_d = nc.dram_tensor("scale", [P, 3 * L], bf16, kind="ExternalInput").ap() \
        .rearrange("p (c l) -> p c l", c=3)
    out_d = nc.dram_tensor("out", [P, 9 * L], bf16, kind="ExternalOutput").ap() \
        .rearrange("p (c l) -> p c l", c=9)

    bounds = []
    t0 = 0
    while t0 < L:
        f = min(F_TILE, L - t0)
        bounds.append((t0, f))
        t0 += f

    # Pin all activations (Ln, Exp, Square, Copy) to the one table that
    # holds them all ("natural_log_exp_and_others"); the default pass
    # ping-pongs Ln/Exp table loads every tile (~1.3us each).  Other list
    # positions are kept (emptied) so act_func_set_id indices stay valid.
    import concourse.bacc as bacc_mod
    from concourse.hw_specs import get_activation_tables

    def _patched_insert_act_table_loads():
        has_activation = any(
            isinstance(i, mybir.InstActivation)
            for b in nc.main_func.blocks
            for i in b.instructions
        )
        if not has_activation:
            return
        keep = "natural_log_exp_and_others"
        tables = [
            (nm, (s if nm == keep else set()))
            for nm, s in get_activation_tables(nc.m.arch).items()
        ]
        assert any(nm == keep and s for nm, s in tables)
        bacc_mod._bass_rust.insert_act_table_loads(nc, tables)

    nc.insert_act_table_loads = _patched_insert_act_table_loads

    with tile.TileContext(nc) as tc, \
            nc.allow_low_precision("bf16 kernel, harness gate is 2e-2"):
        with tc.tile_pool(name="io", bufs=bufs) as io, \
             tc.tile_pool(name="tmp", bufs=bufs) as tp:
            for (t0, F) in bounds:
                rot_t = io.tile([P, 4, F], bf16, tag="rot")
                scl_t = io.tile([P, 3, F], bf16, tag="scl")
                out_t = io.tile([P, 9, F], bf16, tag="out")
                nc.sync.dma_start(out=rot_t, in_=rot_d[:, :, t0:t0 + F])
                nc.sync.dma_start(out=scl_t, in_=scl_d[:, :, t0:t0 + F])

                # ---- fp32 side ----------------------------------------
                sq_t = tp.tile([P, 4, F], f32, tag="sq")
                if act_sq:
                    nc.scalar.activation(
                        out=sq_t.rearrange("p c f -> p (c f)"),
                        in_=rot_t.rearrange("p c f -> p (c f)"),
                        func=Act.Square)
                else:
                    nc.gpsimd.tensor_mul(out=sq_t, in0=rot_t, in1=rot_t)

                s6 = tp.tile([P, 6, F], f32, tag="s6")
                # V2: rows (0,1) = (d+a, b+c)
                eng("V2").tensor_add(out=s6[:, 0:2, :],
                                     in0=sq_t[:, 0:3:2, :],
                                     in1=sq_t[:, 1:4:2, :])
                # V1: rows (4,2) = (d+b, a+c)
                eng("V1").tensor_add(out=s6[:, 4:1:-2, :],
                                     in0=sq_t[:, 0:2, :],
                                     in1=sq_t[:, 2:4, :])
                # G2: row 3 = a+b
                eng("G2").tensor_add(out=s6[:, 3, :],
                                     in0=sq_t[:, 1, :], in1=sq_t[:, 2, :])
                # n2: row 5 = ad + u
                eng("n2").tensor_add(out=s6[:, 5, :],
                                     in0=s6[:, 0, :], in1=s6[:, 1, :])
                n2 = s6[:, 5, :]

                lg = tp.tile([P, F], f32, tag="lg")
                nc.scalar.activation(out=lg, in_=n2, func=Act.Ln)
                tm = tp.tile([P, 3, F], f32, tag="tm")
                eng("tm").tensor_sub(
                    out=tm, in0=scl_t,
                    in1=lg.rearrange("p (c f) -> p c f", c=1)
                         .broadcast_to([P, 3, F]))
                wp = tp.tile([P, 3, F], bf16, tag="wp")
                nc.scalar.activation(out=wp.rearrange("p c f -> p (c f)"),
                                     in_=tm.rearrange("p c f -> p (c f)"),
                                     func=Act.Exp)

                # ---- K/2 assembly (column-major [P, 3j, 3i, F]) -------
                K = tp.tile([P, 3, 3, F], bf16, tag="K")
                K9 = K.rearrange("p j i f -> p (j i) f")
                # diagonal slots (0,4,8): n2/2 - pair
                eng("kd").scalar_tensor_tensor(
                    out=K9[:, 0:9:4, :],
                    in0=n2.rearrange("p (c f) -> p c f", c=1)
                          .broadcast_to([P, 3, F]),
                    scalar=0.5,
                    in1=s6[:, 1:4, :],
                    op0=Alu.mult, op1=Alu.subtract)

                # products
                prodA = tp.tile([P, 3, F], bf16, tag="prodA")
                pa = tp.tile([P, 3, F], bf16, tag="pa")
                eng("P1").tensor_mul(
                    out=prodA, in0=rot_t[:, 1:4, :],
                    in1=rot_t[:, 0:1, :].broadcast_to([P, 3, F]))
                eng("P2").tensor_mul(out=pa[:, 0:3:2, :],
                                     in0=rot_t[:, 1:3, :],
                                     in1=rot_t[:, 2:4, :])
                eng("P3").tensor_mul(out=pa[:, 1, :],
                                     in0=rot_t[:, 1, :], in1=rot_t[:, 3, :])
                pbv = prodA[:, 2::-1, :]          # (kr, jr, ir)

                # off-diagonals: kp = pa+pb -> slots (1,6),(5)
                #                km = pa-pb -> slots (3,2),(7)
                eng("kpA").tensor_add(out=K9[:, 1:7:5, :],
                                      in0=pa[:, 0:2, :], in1=pbv[:, 0:2, :])
                eng("kpB").tensor_add(out=K9[:, 5, :],
                                      in0=pa[:, 2, :], in1=pbv[:, 2, :])
                eng("kmA").tensor_sub(out=K9[:, 3:1:-1, :],
                                      in0=pa[:, 0:2, :], in1=pbv[:, 0:2, :])
                eng("kmB").tensor_sub(out=K9[:, 7, :],
                                      in0=pa[:, 2, :], in1=pbv[:, 2, :])

                # ---- B = Khat * wp (column-uniform broadcast) ---------
                B = tp.tile([P, 3, 3, F], bf16, tag="B")
                eng("rmul").tensor_mul(
                    out=B, in0=K,
                    in1=wp.rearrange("p j (o f) -> p j o f", o=1)
                          .broadcast_to([P, 3, 3, F]))

                # ---- Sigma/4 = B B^T ----------------------------------
                # U rows 0-2: sqB as [i][j]; rows 3-5: T for pairs
                U = tp.tile([P, 6, 3, F], bf16, tag="U")
                nc.scalar.activation(
                    out=U[:, 0:3, :, :].rearrange("p i j f -> p j i f"),
                    in_=B, func=Act.Square)
                for p_, (i_, k_) in enumerate([(0, 1), (0, 2), (1, 2)]):
                    eng(f"T{p_}").tensor_mul(out=U[:, 3 + p_, :, :],
                                             in0=B[:, :, i_, :],
                                             in1=B[:, :, k_, :])
                dd = tp.tile([P, 6, F], bf16, tag="dd")
                eng("ddall").tensor_add(out=dd, in0=U[:, :, 0, :],
                                        in1=U[:, :, 1, :])
                eng("finall").tensor_add(out=out_t[:, 0:6, :], in0=dd,
                                         in1=U[:, :, 2, :])
                # symmetric lower entries
                nc.scalar.copy(out=out_t[:, 6:9, :], in_=out_t[:, 3:6, :])

                nc.sync.dma_start(out=out_d[:, :, t0:t0 + F], in_=out_t)

    nc.compile()
    return nc


def _get_nc():
    if "nc" not in _STATE:
        _STATE["nc"] = _build_kernel()
    return _STATE["nc"]


def kernel(scale: np.ndarray, rot: np.ndarray) -> np.ndarray:
    from concourse.bass_utils import run_bass_kernel_spmd

    scale = np.asarray(scale, dtype=np.float32)
    rot = np.asarray(rot, dtype=np.float32)

    nc = _get_nc()

    in_maps = []
    for c in range(N_CORES):
        s = np.zeros((N_PAD, 3), np.float32)
        s[:N_PER_CORE] = scale[c * N_PER_CORE:(c + 1) * N_PER_CORE]
        r = np.zeros((N_PAD, 4), np.float32)
        r[:N_PER_CORE] = rot[c * N_PER_CORE:(c + 1) * N_PER_CORE]
        r[N_PER_CORE:, 0] = 1.0
        sb = np.ascontiguousarray(
            s.reshape(P, L, 3).transpose(0, 2, 1)).astype(BF16)
        rb = np.ascontiguousarray(
            r.reshape(P, L, 4).transpose(0, 2, 1)).astype(BF16)
        in_maps.append({
            "scale": sb.reshape(P, 3 * L),
            "rot": rb.reshape(P, 4 * L),
        })

    res = run_bass_kernel_spmd(nc, in_maps, core_ids=list(range(N_CORES)))

    # device channel order: [S00,S11,S22,S01,S02,S12,S10,S20,S21] (all /4)
    perm = [0, 3, 4, 6, 1, 5, 7, 8, 2]
    out = np.empty((N_TOTAL, 9), np.float32)
    for c in range(N_CORES):
        o = res.results[c]["out"].reshape(P, 9, L)[:, perm, :].transpose(0, 2, 1)
        out[c * N_PER_CORE:(c + 1) * N_PER_CORE] = (
            o.reshape(N_PAD, 9)[:N_PER_CORE].astype(np.float32))
    out *= 4.0
    return out.reshape(N_TOTAL, 3, 3)
